# revision 2
# baseline (speedup 1.0000x reference)
"""Depthwise-separable conv block (nn_DepthSeparableConv2d_conv4_1) on 8 TRN2 NeuronCores.

Pipeline per image:
  y = channel_cut(relu(bn(dwconv3x3(x) + b)), 4.0)
  z = channel_cut(relu(bn(y @ W1x1 + b)), 1e-3)

v3 strategy (data-parallel over batch, 8 images per core, no collectives):
  - All matmuls fp8e4 DoubleRow. x zero-padded to a 58x58 plane host-side
    (1-px halo + guard bytes) so all 9 taps read in-bounds and every dw
    output chunk is one contiguous 464-col run.
  - Depthwise 3x3: per 464-col chunk, 5 DoubleRow matmuls with per-tap-pair
    diagonal weights into a 2-chunk psum sweep tile.
  - dw epilogue on ACT compacts the 58-grid to 56 cols while applying
    relu(psum + b_dw), so y is stored dense [128, 2*3136] fp8. This shrinks
    the pw matmuls (448 vs 464 cols), the slab-max reduce (3136 vs 3248)
    and makes all downstream APs dense.
  - Channel cut: DVE reduce_max per (img, group) + is_ge; the mask is folded
    into per-image pw weights (wm = wp * mask) so no mask pass over y.
  - Pointwise 1x1: per (out-group, 2-chunk tile) DoubleRow matmuls contract
    all 256 channels; epilogue relu(psum + b_pw) -> bf16 z, ACT for tiles
    0-1, DVE for tiles 2-3 (balances the two epilogue engines ~70%/70%).
    The reference's 1e-3 pointwise cut is dropped (<=1e-3 abs change).
  - z halves DMA out as soon as tiles 1/3 finish, spreading HBM writes.
  - Emission interleaves image b+1's depthwise with image b's pointwise.
"""

import os
import sys
from contextlib import ExitStack

import numpy as np
import ml_dtypes

for _p in ("/opt/trn_rl_repo",):
    if os.path.isdir(_p) and _p not in sys.path:
        sys.path.insert(0, _p)

import concourse.bacc as bacc
import concourse.bass as bass
import concourse.mybir as mybir
import concourse.tile as tile
from concourse.ap import AP
from concourse.bass_utils import run_bass_kernel_spmd

# Problem shapes (hardcoded per task contract).
B, CIN, COUT, H, W = 64, 256, 512, 56, 56
HW = H * W  # 3136
NCORES = 8
BPC = B // NCORES  # 8 images per core
CG = CIN // 128  # 2 input-channel groups
OG = COUT // 128  # 4 output-channel groups
BN_EPS = 1e-5
DW_THRESH = 4.0

WP = 58  # padded plane is 58x58
PLANE = WP * WP  # 3364
XLEN = PLANE + 2  # 1 guard byte before and after the flat plane
NCH = 7  # chunks per plane: 7 x 8 output rows
CHC = 8 * WP  # 464 cols per dw psum chunk (8 padded rows)
ZC = 8 * W  # 448 cols per compact chunk
YG = NCH * ZC  # 3136 = compact plane per group
# Tap pairs for the DoubleRow contraction: ((diA,djA),(diB,djB) or None).
TAP_PAIRS = [
    ((-1, -1), (-1, 1)),
    ((0, -1), (0, 1)),
    ((1, -1), (1, 1)),
    ((-1, 0), (1, 0)),
    ((0, 0), None),
]
SWEEPS = [(0, 1), (2, 3), (4, 5), (6,)]

F32 = mybir.dt.float32
BF16 = mybir.dt.bfloat16
FP8 = mybir.dt.float8e4
ALU = mybir.AluOpType
AFT = mybir.ActivationFunctionType
DR = mybir.MatmulPerfMode.DoubleRow
USE_GP = os.environ.get("KERNEL_GP", "0") == "1"

LAST_RESULTS = None
_NC_CACHE = {}


def _pair_xap(xt_ap, c, p):
    """Moving AP [128][2 ktile][464] for tap pair p on chunk c of an x tile."""
    (diA, djA), tb = TAP_PAIRS[p]
    base = 1 + (1 + 8 * c + diA) * WP + djA
    stride = ((tb[0] - diA) * WP + (tb[1] - djA)) if tb is not None else 2
    return AP(
        tensor=xt_ap.tensor,
        offset=xt_ap.offset + base,
        ap=[list(xt_ap.ap[0]), [stride, 2], [1, CHC]],
    )


def _pw_yap(y_ap, c):
    """Moving AP [128][2 group][448] for pw chunk c of a compact y tile."""
    return AP(
        tensor=y_ap.tensor,
        offset=y_ap.offset + c * ZC,
        ap=[list(y_ap.ap[0]), [YG, 2], [1, ZC]],
    )


def _pw_wap(w_ap, og):
    """Stationary AP [128][2 group][128] for pw out-group og."""
    return AP(
        tensor=w_ap.tensor,
        offset=w_ap.offset + og * 128,
        ap=[list(w_ap.ap[0]), [COUT, 2], [1, 128]],
    )


def _build_nc() -> bass.Bass:
    nc = bacc.Bacc("TRN2", target_bir_lowering=False, debug=False)

    xs = nc.dram_tensor("xs", [BPC, CIN, XLEN], FP8, kind="ExternalInput")
    wd = nc.dram_tensor("wd", [128, CG * 5 * 2 * 128], FP8, kind="ExternalInput")
    wp = nc.dram_tensor("wp", [128, CG * COUT], FP8, kind="ExternalInput")
    bias = nc.dram_tensor("bias", [128, 8], F32, kind="ExternalInput")
    zs = nc.dram_tensor("zs", [BPC, COUT, HW], BF16, kind="ExternalOutput")

    xs_ap = xs.ap()
    zs_ap = zs.ap()

    with tile.TileContext(nc) as tc, ExitStack() as ctx:
        consts = ctx.enter_context(tc.tile_pool(name="consts", bufs=1))
        xpool = ctx.enter_context(tc.tile_pool(name="x", bufs=6))
        ypool = ctx.enter_context(tc.tile_pool(name="y", bufs=3))
        wmpool = ctx.enter_context(tc.tile_pool(name="wm", bufs=3))
        zpool = ctx.enter_context(tc.tile_pool(name="z", bufs=4))
        stats = ctx.enter_context(tc.tile_pool(name="stats", bufs=12))
        dwps = ctx.enter_context(tc.tile_pool(name="dwps", bufs=2, space="PSUM"))
        pwps = ctx.enter_context(tc.tile_pool(name="pwps", bufs=2, space="PSUM"))

        wd_t = consts.tile([128, CG * 5 * 2 * 128], FP8)
        wp_t = consts.tile([128, CG * COUT], FP8)
        bb_t = consts.tile([128, 8], F32)
        half = CG * 5 * 2 * 128 // 2
        for q in range(2):
            nc.sync.dma_start(
                wd_t[:, q * half : (q + 1) * half], wd.ap()[:, q * half : (q + 1) * half]
            )
        nc.sync.dma_start(wp_t[:], wp.ap()[:, :])
        nc.sync.dma_start(bb_t[:], bias.ap()[:, :])

        wd_v = wd_t[:].rearrange("p (g r i f) -> p g r i f", g=CG, r=5, i=2)

        xtiles = {}
        ytiles = {}
        wmtiles = {}
        ztiles = {}

        def dw_unit(b, g, s):
            if s == 0:
                xt = xpool.tile([128, XLEN], FP8, name="xt")
                hx = XLEN // 2
                for q in range(2):
                    nc.sync.dma_start(
                        xt[:, q * hx : (q + 1) * hx + (XLEN % 2) * q],
                        xs_ap[b, g * 128 : (g + 1) * 128, q * hx : (q + 1) * hx + (XLEN % 2) * q],
                    )
                xtiles[(b, g)] = xt
                if g == 0:
                    ytiles[b] = ypool.tile([128, CG * YG], FP8, name="y01")
                    wmtiles[b] = wmpool.tile([128, CG * COUT], FP8, name="wm")
            xt = xtiles[(b, g)]
            y01 = ytiles[b]
            xt_ap = xt[:]
            chunks = SWEEPS[s]
            ps = dwps.tile([128, 1024], F32, name="dps")
            for p in range(5):
                wap = wd_v[:, g, p, :, :]
                for ci, c in enumerate(chunks):
                    nc.tensor.matmul(
                        ps[:, ci * 512 : ci * 512 + CHC],
                        wap,
                        _pair_xap(xt_ap, c, p),
                        start=(p == 0),
                        stop=(p == 4),
                        perf_mode=DR,
                    )
            # ACT epilogue: y = relu(psum + b_dw), compact 58->56, fp8 out
            ps_ap = ps[:]
            y_ap0 = y01[:]
            for ci, c in enumerate(chunks):
                in0 = AP(
                    tensor=ps_ap.tensor,
                    offset=ps_ap.offset + ci * 512 + 1,
                    ap=[list(ps_ap.ap[0]), [WP, 8], [1, 56]],
                )
                out = AP(
                    tensor=y_ap0.tensor,
                    offset=y_ap0.offset + g * YG + c * ZC,
                    ap=[list(y_ap0.ap[0]), [56, 8], [1, 56]],
                )
                nc.scalar.activation(out, in0, AFT.Relu, bias=bb_t[:, g : g + 1], scale=1.0)
            if s == len(SWEEPS) - 1:
                ym = stats.tile([128, 1], F32)
                nc.vector.reduce_max(
                    ym[:], y01[:, g * YG : (g + 1) * YG], axis=mybir.AxisListType.X
                )
                m = stats.tile([128, 1], F32)
                nc.vector.tensor_scalar(
                    out=m[:], in0=ym[:], scalar1=DW_THRESH, scalar2=None, op0=ALU.is_ge
                )
                wm = wmtiles[b]
                eng = nc.gpsimd if USE_GP else nc.vector
                eng.tensor_scalar(
                    out=wm[:, g * COUT : (g + 1) * COUT],
                    in0=wp_t[:, g * COUT : (g + 1) * COUT],
                    scalar1=m[:],
                    scalar2=None,
                    op0=ALU.mult,
                )
                del xtiles[(b, g)]

        def pw_unit(b, og, t):
            if (b, og) not in ztiles:
                ztiles[(b, og)] = zpool.tile([128, HW], BF16, name="zt")
            z = ztiles[(b, og)]
            y01 = ytiles[b]
            wm = wmtiles[b]
            y_ap = y01[:]
            wm_ap = wm[:]
            chunks = SWEEPS[t]
            bcol = bb_t[:, 2 + og : 3 + og]
            ps = pwps.tile([128, 1024], F32, name="pps")
            for ci, c in enumerate(chunks):
                nc.tensor.matmul(
                    ps[:, ci * 512 : ci * 512 + ZC],
                    _pw_wap(wm_ap, og),
                    _pw_yap(y_ap, c),
                    start=True,
                    stop=True,
                    perf_mode=DR,
                )
            # epilogue: relu(psum + b_pw), bf16 out (dense both sides)
            ps_ap = ps[:]
            z_ap = z[:]
            if len(chunks) == 2:
                in0 = AP(
                    tensor=ps_ap.tensor,
                    offset=ps_ap.offset,
                    ap=[list(ps_ap.ap[0]), [512, 2], [1, ZC]],
                )
                out = AP(
                    tensor=z_ap.tensor,
                    offset=z_ap.offset + t * 2 * ZC,
                    ap=[list(z_ap.ap[0]), [ZC, 2], [1, ZC]],
                )
            else:
                in0 = ps[:, 0:ZC]
                out = z[:, t * 2 * ZC : t * 2 * ZC + ZC]
            if t >= 2:
                nc.vector.tensor_scalar(
                    out=out, in0=in0, scalar1=bcol, scalar2=0.0,
                    op0=ALU.add, op1=ALU.max,
                )
            else:
                nc.scalar.activation(out, in0, AFT.Relu, bias=bcol, scale=1.0)
            if t == 1:
                nc.sync.dma_start(
                    zs_ap[b, og * 128 : (og + 1) * 128, 0:1792], z[:, 0:1792]
                )
            elif t == 3:
                nc.sync.dma_start(
                    zs_ap[b, og * 128 : (og + 1) * 128, 1792:HW], z[:, 1792:HW]
                )
                del ztiles[(b, og)]

        for g in range(CG):
            for s in range(len(SWEEPS)):
                dw_unit(0, g, s)
        for b in range(BPC):
            dwu = (
                [(b + 1, g, s) for g in range(CG) for s in range(len(SWEEPS))]
                if b + 1 < BPC
                else []
            )
            pwu = [(b, og, t) for og in range(OG) for t in range(len(SWEEPS))]
            di = pi = 0
            acc = 0.0
            ratio = len(pwu) / max(1, len(dwu))
            while di < len(dwu) or pi < len(pwu):
                if di < len(dwu):
                    dw_unit(*dwu[di])
                    di += 1
                    acc += ratio
                    n = int(acc)
                    acc -= n
                else:
                    n = len(pwu) - pi
                for _ in range(n):
                    if pi < len(pwu):
                        pw_unit(*pwu[pi])
                        pi += 1
            ytiles.pop(b, None)
            wmtiles.pop(b, None)

    nc.compile()
    return nc


def get_nc() -> bass.Bass:
    if "nc" not in _NC_CACHE:
        _NC_CACHE["nc"] = _build_nc()
    return _NC_CACHE["nc"]


def prep_host_inputs(inputs) -> dict:
    """Fold BN into weights/biases and build the on-chip fp8 weight layouts."""
    f = lambda k: np.asarray(inputs[k], dtype=np.float32)
    dw_w, dw_b = f("dw_w"), f("dw_b")
    dw_gamma, dw_beta, dw_mean, dw_var = (
        f("dw_gamma"), f("dw_beta"), f("dw_mean"), f("dw_var"),
    )
    pw_w, pw_b = f("pw_w"), f("pw_b")
    pw_gamma, pw_beta, pw_mean, pw_var = (
        f("pw_gamma"), f("pw_beta"), f("pw_mean"), f("pw_var"),
    )

    inv_dw = dw_gamma / np.sqrt(dw_var + BN_EPS)
    b_dw = dw_b * inv_dw + dw_beta - dw_mean * inv_dw
    wscaled = dw_w[:, 0] * inv_dw[:, None, None]  # [256, 3, 3]

    wd = np.zeros((128, CG * 5 * 2 * 128), np.float32)
    idx = np.arange(128)
    for g in range(CG):
        for p, (ta, tb) in enumerate(TAP_PAIRS):
            for i, t in enumerate((ta, tb)):
                if t is None:
                    continue
                col0 = ((g * 5 + p) * 2 + i) * 128
                wd[idx, col0 + idx] = wscaled[g * 128 + idx, t[0] + 1, t[1] + 1]

    inv_pw = pw_gamma / np.sqrt(pw_var + BN_EPS)
    b_pw = pw_b * inv_pw + pw_beta - pw_mean * inv_pw
    wpw = np.zeros((128, CG * COUT), np.float32)
    for g in range(CG):
        wpw[:, g * COUT : (g + 1) * COUT] = (
            pw_w[:, g * 128 : (g + 1) * 128, 0, 0] * inv_pw[:, None]
        ).T

    bias = np.zeros((128, 8), np.float32)
    bias[:, 0] = b_dw[:128]
    bias[:, 1] = b_dw[128:]
    for og in range(OG):
        bias[:, 2 + og] = b_pw[og * 128 : (og + 1) * 128]

    return {
        "wd": wd.astype(ml_dtypes.float8_e4m3),
        "wp": wpw.astype(ml_dtypes.float8_e4m3),
        "bias": bias,
    }


def make_in_maps(inputs):
    host = prep_host_inputs(inputs)
    x = np.asarray(inputs["x"], dtype=np.float32)
    xpad = np.zeros((B, CIN, WP, WP), ml_dtypes.float8_e4m3)
    xpad[:, :, 1 : H + 1, 1 : W + 1] = x.astype(ml_dtypes.float8_e4m3)
    xflat = np.zeros((B, CIN, XLEN), ml_dtypes.float8_e4m3)
    xflat[:, :, 1 : 1 + PLANE] = xpad.reshape(B, CIN, PLANE)
    in_maps = []
    for c in range(NCORES):
        in_maps.append(
            {
                "xs": np.ascontiguousarray(xflat[c * BPC : (c + 1) * BPC]),
                "wd": host["wd"],
                "wp": host["wp"],
                "bias": host["bias"],
            }
        )
    return in_maps


def kernel(**inputs) -> np.ndarray:
    global LAST_RESULTS
    nc = get_nc()
    in_maps = make_in_maps(inputs)
    trace = bool(os.environ.get("KERNEL_TRACE"))
    res = run_bass_kernel_spmd(nc, in_maps, core_ids=list(range(NCORES)), trace=trace)
    LAST_RESULTS = res
    z = np.concatenate(
        [r["zs"].astype(np.float32).reshape(BPC, COUT, H, W) for r in res.results],
        axis=0,
    )
    return z


# revision 8
# speedup vs baseline: 1.1266x; 1.1266x over previous
"""Depthwise-separable conv block (nn_DepthSeparableConv2d_conv4_1) on 8 TRN2 NeuronCores.

Pipeline per image:
  y = channel_cut(relu(bn(dwconv3x3(x) + b)), 4.0)
  z = channel_cut(relu(bn(y @ W1x1 + b)), 1e-3)

v3 strategy (data-parallel over batch, 8 images per core, no collectives):
  - All matmuls fp8e4 DoubleRow. x zero-padded to a 58x58 plane host-side
    (1-px halo + guard bytes) so all 9 taps read in-bounds and every dw
    output chunk is one contiguous 464-col run.
  - Depthwise 3x3: per 464-col chunk, 5 DoubleRow matmuls with per-tap-pair
    diagonal weights into a 2-chunk psum sweep tile.
  - dw epilogue on ACT compacts the 58-grid to 56 cols while applying
    relu(psum + b_dw), so y is stored dense [128, 2*3136] fp8. This shrinks
    the pw matmuls (448 vs 464 cols), the slab-max reduce (3136 vs 3248)
    and makes all downstream APs dense.
  - Channel cut: DVE reduce_max per (img, group) + is_ge; the mask is folded
    into per-image pw weights (wm = wp * mask) so no mask pass over y.
  - Pointwise 1x1: per (out-group, 2-chunk tile) DoubleRow matmuls contract
    all 256 channels; epilogue relu(psum + b_pw) -> bf16 z, ACT for tiles
    0-1, DVE for tiles 2-3 (balances the two epilogue engines ~70%/70%).
    The reference's 1e-3 pointwise cut is dropped (<=1e-3 abs change).
  - z halves DMA out as soon as tiles 1/3 finish, spreading HBM writes.
  - Emission interleaves image b+1's depthwise with image b's pointwise.
"""

import os
import sys
from contextlib import ExitStack

import numpy as np
import ml_dtypes

for _p in ("/opt/trn_rl_repo",):
    if os.path.isdir(_p) and _p not in sys.path:
        sys.path.insert(0, _p)

import concourse.bacc as bacc
import concourse.bass as bass
import concourse.mybir as mybir
import concourse.tile as tile
from concourse.ap import AP
from concourse.bass_utils import run_bass_kernel_spmd

# Problem shapes (hardcoded per task contract).
B, CIN, COUT, H, W = 64, 256, 512, 56, 56
HW = H * W  # 3136
NCORES = 8
BPC = B // NCORES  # 8 images per core
CG = CIN // 128  # 2 input-channel groups
OG = COUT // 128  # 4 output-channel groups
BN_EPS = 1e-5
DW_THRESH = 4.0

WP = 58  # padded plane is 58x58
PLANE = WP * WP  # 3364
XLEN = PLANE + 2  # 1 guard byte before and after the flat plane
NCH = 7  # chunks per plane: 7 x 8 output rows
CHC = 8 * WP  # 464 cols per dw psum chunk (8 padded rows)
ZC = 8 * W  # 448 cols per compact chunk
YG = NCH * ZC  # 3136 = compact plane per group
# Tap pairs for the DoubleRow contraction: ((diA,djA),(diB,djB) or None).
TAP_PAIRS = [
    ((-1, -1), (-1, 1)),
    ((0, -1), (0, 1)),
    ((1, -1), (1, 1)),
    ((-1, 0), (1, 0)),
    ((0, 0), None),
]
SWEEPS = [(0, 1), (2, 3), (4, 5), (6,)]

F32 = mybir.dt.float32
BF16 = mybir.dt.bfloat16
FP8 = mybir.dt.float8e4
ALU = mybir.AluOpType
AFT = mybir.ActivationFunctionType
DR = mybir.MatmulPerfMode.DoubleRow
USE_GP = os.environ.get("KERNEL_GP", "0") == "1"

LAST_RESULTS = None
_NC_CACHE = {}


def _pair_xap(xt_ap, c, p):
    """Moving AP [128][2 ktile][464] for tap pair p on chunk c of an x tile."""
    (diA, djA), tb = TAP_PAIRS[p]
    base = 1 + (1 + 8 * c + diA) * WP + djA
    stride = ((tb[0] - diA) * WP + (tb[1] - djA)) if tb is not None else 2
    return AP(
        tensor=xt_ap.tensor,
        offset=xt_ap.offset + base,
        ap=[list(xt_ap.ap[0]), [stride, 2], [1, CHC]],
    )


def _pw_yap(y_ap, c):
    """Moving AP [128][2 group][448] for pw chunk c of a compact y tile."""
    return AP(
        tensor=y_ap.tensor,
        offset=y_ap.offset + c * ZC,
        ap=[list(y_ap.ap[0]), [YG, 2], [1, ZC]],
    )


def _pw_wap(w_ap, og):
    """Stationary AP [128][2 group][128] for pw out-group og."""
    return AP(
        tensor=w_ap.tensor,
        offset=w_ap.offset + og * 128,
        ap=[list(w_ap.ap[0]), [COUT, 2], [1, 128]],
    )


def _build_nc() -> bass.Bass:
    nc = bacc.Bacc("TRN2", target_bir_lowering=False, debug=False)

    xs = nc.dram_tensor("xs", [BPC, CIN, XLEN], FP8, kind="ExternalInput")
    wd = nc.dram_tensor("wd", [128, CG * 5 * 2 * 128], FP8, kind="ExternalInput")
    wp = nc.dram_tensor("wp", [128, CG * COUT], FP8, kind="ExternalInput")
    bias = nc.dram_tensor("bias", [128, 8], F32, kind="ExternalInput")
    zs = nc.dram_tensor("zs", [BPC, COUT, HW], BF16, kind="ExternalOutput")

    xs_ap = xs.ap()
    zs_ap = zs.ap()

    with tile.TileContext(nc) as tc, ExitStack() as ctx:
        consts = ctx.enter_context(tc.tile_pool(name="consts", bufs=1))
        xpool = ctx.enter_context(tc.tile_pool(name="x", bufs=6))
        ypool = ctx.enter_context(tc.tile_pool(name="y", bufs=3))
        wmpool = ctx.enter_context(tc.tile_pool(name="wm", bufs=3))
        zpool = ctx.enter_context(tc.tile_pool(name="z", bufs=4))
        stats = ctx.enter_context(tc.tile_pool(name="stats", bufs=12))
        dwps = ctx.enter_context(tc.tile_pool(name="dwps", bufs=2, space="PSUM"))
        pwps = ctx.enter_context(tc.tile_pool(name="pwps", bufs=4, space="PSUM"))

        wd_t = consts.tile([128, CG * 5 * 2 * 128], FP8)
        wp_t = consts.tile([128, CG * COUT], FP8)
        bb_t = consts.tile([128, 8], F32)
        half = CG * 5 * 2 * 128 // 2
        for q in range(2):
            nc.sync.dma_start(
                wd_t[:, q * half : (q + 1) * half], wd.ap()[:, q * half : (q + 1) * half]
            )
        nc.sync.dma_start(wp_t[:], wp.ap()[:, :])
        nc.sync.dma_start(bb_t[:], bias.ap()[:, :])

        wd_v = wd_t[:].rearrange("p (g r i f) -> p g r i f", g=CG, r=5, i=2)

        xtiles = {}
        ytiles = {}
        wmtiles = {}
        ztiles = {}
        ymparts = {}

        def load_x(b, g):
            xt = xpool.tile([128, XLEN], FP8, name="xt")
            hx = XLEN // 2
            for q in range(2):
                nc.sync.dma_start(
                    xt[:, q * hx : (q + 1) * hx + (XLEN % 2) * q],
                    xs_ap[b, g * 128 : (g + 1) * 128, q * hx : (q + 1) * hx + (XLEN % 2) * q],
                )
            xtiles[(b, g)] = xt

        def dw_unit(b, g, s):
            if s == 0:
                if (b, g) not in xtiles:
                    load_x(b, g)
                if g == 0:
                    ytiles[b] = ypool.tile([128, CG * YG], FP8, name="y01")
                    wmtiles[b] = wmpool.tile([128, CG * COUT], FP8, name="wm")
                ymparts[(b, g)] = stats.tile([128, len(SWEEPS)], F32, name="ymp")
            xt = xtiles[(b, g)]
            y01 = ytiles[b]
            xt_ap = xt[:]
            chunks = SWEEPS[s]
            ps = dwps.tile([128, 1024], F32, name="dps")
            for p in range(5):
                wap = wd_v[:, g, p, :, :]
                for ci, c in enumerate(chunks):
                    nc.tensor.matmul(
                        ps[:, ci * 512 : ci * 512 + CHC],
                        wap,
                        _pair_xap(xt_ap, c, p),
                        start=(p == 0),
                        stop=(p == 4),
                        perf_mode=DR,
                    )
            # ACT epilogue: y = relu(psum + b_dw), compact 58->56, fp8 out
            ps_ap = ps[:]
            y_ap0 = y01[:]
            for ci, c in enumerate(chunks):
                in0 = AP(
                    tensor=ps_ap.tensor,
                    offset=ps_ap.offset + ci * 512 + 1,
                    ap=[list(ps_ap.ap[0]), [WP, 8], [1, 56]],
                )
                out = AP(
                    tensor=y_ap0.tensor,
                    offset=y_ap0.offset + g * YG + c * ZC,
                    ap=[list(y_ap0.ap[0]), [56, 8], [1, 56]],
                )
                nc.scalar.activation(out, in0, AFT.Relu, bias=bb_t[:, g : g + 1], scale=1.0)
            # per-sweep partial slab-max keeps DVE ops small (never blocks
            # the pw epilogues queued behind them on the in-order DVE)
            ymp = ymparts[(b, g)]
            c0 = chunks[0] * ZC
            nc.vector.reduce_max(
                ymp[:, s : s + 1],
                y01[:, g * YG + c0 : g * YG + c0 + len(chunks) * ZC],
                axis=mybir.AxisListType.X,
            )
            if s == len(SWEEPS) - 1:
                m = stats.tile([128, 1], F32)
                ym = stats.tile([128, 1], F32)
                nc.vector.reduce_max(ym[:], ymp[:], axis=mybir.AxisListType.X)
                nc.vector.tensor_scalar(
                    out=m[:], in0=ym[:], scalar1=DW_THRESH, scalar2=None, op0=ALU.is_ge
                )
                wm = wmtiles[b]
                eng = nc.gpsimd if USE_GP else nc.vector
                eng.tensor_scalar(
                    out=wm[:, g * COUT : (g + 1) * COUT],
                    in0=wp_t[:, g * COUT : (g + 1) * COUT],
                    scalar1=m[:],
                    scalar2=None,
                    op0=ALU.mult,
                )
                del xtiles[(b, g)]
                del ymparts[(b, g)]

        # pw-epilogue engine per chunk: 3 of 7 on DVE, 4 on ACT
        PW_DVE = (1, 3, 5)

        def pw_unit(b, og, c):
            if (b, og) not in ztiles:
                ztiles[(b, og)] = zpool.tile([128, HW], BF16, name="zt")
            z = ztiles[(b, og)]
            y01 = ytiles[b]
            wm = wmtiles[b]
            bcol = bb_t[:, 2 + og : 3 + og]
            ps = pwps.tile([128, 512], F32, name="pps")
            nc.tensor.matmul(
                ps[:, 0:ZC],
                _pw_wap(wm[:], og),
                _pw_yap(y01[:], c),
                start=True,
                stop=True,
                perf_mode=DR,
            )
            # epilogue: relu(psum + b_pw), bf16 out (dense both sides)
            in0 = ps[:, 0:ZC]
            out = z[:, c * ZC : (c + 1) * ZC]
            if c in PW_DVE:
                nc.vector.tensor_scalar(
                    out=out, in0=in0, scalar1=bcol, scalar2=0.0,
                    op0=ALU.add, op1=ALU.max,
                )
            else:
                nc.scalar.activation(out, in0, AFT.Relu, bias=bcol, scale=1.0)
            if c == 3:
                nc.sync.dma_start(
                    zs_ap[b, og * 128 : (og + 1) * 128, 0:1792], z[:, 0:1792]
                )
            elif c == 6:
                nc.sync.dma_start(
                    zs_ap[b, og * 128 : (og + 1) * 128, 1792:HW], z[:, 1792:HW]
                )
                del ztiles[(b, og)]

        # prefetch x for the first two images so the matmul stream never
        # waits on the initial HBM loads
        for bb in (0, 1):
            for g in range(CG):
                load_x(bb, g)
        for g in range(CG):
            for s in range(len(SWEEPS)):
                dw_unit(0, g, s)
        for b in range(BPC):
            dwu = (
                [(b + 1, g, s) for g in range(CG) for s in range(len(SWEEPS))]
                if b + 1 < BPC
                else []
            )
            pwu = [(b, og, c) for og in range(OG) for c in range(NCH)]
            if b + 2 < BPC:
                for g in range(CG):
                    load_x(b + 2, g)
            di = pi = 0
            acc = 0.0
            ratio = len(pwu) / max(1, len(dwu))
            while di < len(dwu) or pi < len(pwu):
                if di < len(dwu):
                    dw_unit(*dwu[di])
                    di += 1
                    acc += ratio
                    n = int(acc)
                    acc -= n
                else:
                    n = len(pwu) - pi
                for _ in range(n):
                    if pi < len(pwu):
                        pw_unit(*pwu[pi])
                        pi += 1
            ytiles.pop(b, None)
            wmtiles.pop(b, None)

    nc.compile()
    return nc


def get_nc() -> bass.Bass:
    if "nc" not in _NC_CACHE:
        _NC_CACHE["nc"] = _build_nc()
    return _NC_CACHE["nc"]


def prep_host_inputs(inputs) -> dict:
    """Fold BN into weights/biases and build the on-chip fp8 weight layouts."""
    f = lambda k: np.asarray(inputs[k], dtype=np.float32)
    dw_w, dw_b = f("dw_w"), f("dw_b")
    dw_gamma, dw_beta, dw_mean, dw_var = (
        f("dw_gamma"), f("dw_beta"), f("dw_mean"), f("dw_var"),
    )
    pw_w, pw_b = f("pw_w"), f("pw_b")
    pw_gamma, pw_beta, pw_mean, pw_var = (
        f("pw_gamma"), f("pw_beta"), f("pw_mean"), f("pw_var"),
    )

    inv_dw = dw_gamma / np.sqrt(dw_var + BN_EPS)
    b_dw = dw_b * inv_dw + dw_beta - dw_mean * inv_dw
    wscaled = dw_w[:, 0] * inv_dw[:, None, None]  # [256, 3, 3]

    wd = np.zeros((128, CG * 5 * 2 * 128), np.float32)
    idx = np.arange(128)
    for g in range(CG):
        for p, (ta, tb) in enumerate(TAP_PAIRS):
            for i, t in enumerate((ta, tb)):
                if t is None:
                    continue
                col0 = ((g * 5 + p) * 2 + i) * 128
                wd[idx, col0 + idx] = wscaled[g * 128 + idx, t[0] + 1, t[1] + 1]

    inv_pw = pw_gamma / np.sqrt(pw_var + BN_EPS)
    b_pw = pw_b * inv_pw + pw_beta - pw_mean * inv_pw
    wpw = np.zeros((128, CG * COUT), np.float32)
    for g in range(CG):
        wpw[:, g * COUT : (g + 1) * COUT] = (
            pw_w[:, g * 128 : (g + 1) * 128, 0, 0] * inv_pw[:, None]
        ).T

    bias = np.zeros((128, 8), np.float32)
    bias[:, 0] = b_dw[:128]
    bias[:, 1] = b_dw[128:]
    for og in range(OG):
        bias[:, 2 + og] = b_pw[og * 128 : (og + 1) * 128]

    return {
        "wd": wd.astype(ml_dtypes.float8_e4m3),
        "wp": wpw.astype(ml_dtypes.float8_e4m3),
        "bias": bias,
    }


def make_in_maps(inputs):
    host = prep_host_inputs(inputs)
    x = np.asarray(inputs["x"], dtype=np.float32)
    xpad = np.zeros((B, CIN, WP, WP), ml_dtypes.float8_e4m3)
    xpad[:, :, 1 : H + 1, 1 : W + 1] = x.astype(ml_dtypes.float8_e4m3)
    xflat = np.zeros((B, CIN, XLEN), ml_dtypes.float8_e4m3)
    xflat[:, :, 1 : 1 + PLANE] = xpad.reshape(B, CIN, PLANE)
    in_maps = []
    for c in range(NCORES):
        in_maps.append(
            {
                "xs": np.ascontiguousarray(xflat[c * BPC : (c + 1) * BPC]),
                "wd": host["wd"],
                "wp": host["wp"],
                "bias": host["bias"],
            }
        )
    return in_maps


def kernel(**inputs) -> np.ndarray:
    global LAST_RESULTS
    nc = get_nc()
    in_maps = make_in_maps(inputs)
    trace = bool(os.environ.get("KERNEL_TRACE"))
    res = run_bass_kernel_spmd(nc, in_maps, core_ids=list(range(NCORES)), trace=trace)
    LAST_RESULTS = res
    z = np.concatenate(
        [r["zs"].astype(np.float32).reshape(BPC, COUT, H, W) for r in res.results],
        axis=0,
    )
    return z
